# revision 22
# baseline (speedup 1.0000x reference)
"""Trainium2 Bass kernel for nn_DecoderLayer (B=4,S=2048,D=1024,H=16,FF=4096).

Sharding: 16 heads / 8 cores = 2 heads per core (head parallel) for
attention; one ReduceScatter per batch (4 phases) of the head-summed
attention output; token-parallel LN+FFN on each core's 256-token slice
per phase; host concatenates the shards.

Numerics:
- x pre-transposed on host to [d, tok] in bf16 (QK proj) and fp8e4m3
  (V proj) — no on-device transposes for attention.
- QK proj bf16; scores bf16 with the padding bias folded in as a 65th
  contraction row (kv row 64 = pad - SHIFT, qv row 64 = 1), so the exp
  activation needs no bias and covers both heads in one instruction.
- exp outputs fp8e4m3 (SHIFT=3 keeps exp(max logit) ~28 << 240).
- V proj fp8e4m3 DoubleRow (Wv prescaled by 32 on host, rescaled at
  eviction); v stores NO bias — bias is added after softmax since
  sum(p)=1; a 1025th ones-column in v makes P@V also produce the
  softmax denominator in psum column 1024.
- P@V fp8e4m3 DoubleRow; per-head outputs combined in SBUF
  (ops_h0*rd0 + bvsum, then + ops_h1*rd1) — single f32 write per block.
- FFN bf16 as before; w2 resident in SBUF; ff2 accumulates all 32
  f-blocks in one PSUM tile.
"""
import numpy as np
import ml_dtypes
from collections import deque
from contextlib import ExitStack

import concourse.bass as bass
import concourse.tile as tile
from concourse import bacc, mybir

dt = mybir.dt
F32 = dt.float32
BF16 = dt.bfloat16
F8 = dt.float8e4
AF = mybir.ActivationFunctionType
ALU = mybir.AluOpType
AX = mybir.AxisListType
DR = mybir.MatmulPerfMode.DoubleRow

KD = 64
EPS = 1e-5
NEG = -30000.0
SHIFT = 3.0
SV = 32.0
SX = 4.0

CFG_MAIN = dict(B=4, S=2048, D=1024, FF=4096, ncores=8, HPC=2)


def build_nc(B, S, D, FF, ncores, HPC):
    DC = D // 128            # 8 d-chunks
    TB = S // 128            # 16 t-blocks per batch
    IC = S // 256            # 8 query chunks of 256
    NP = TB // 2             # 8 tb-pairs per batch
    EC = D // 512            # 2
    FB = FF // 128           # 32
    CH = S // 512            # 4 512-token chunks per batch
    PH = B                   # 4 RS phases, one per batch
    SLICE = 257              # 256 tokens + 1 vmean row
    QT = 256                 # tokens per FFN quarter per core
    TBH = QT // 128          # 2
    VW = 2 * NP * 1025       # v8 free size per head

    nc = bacc.Bacc("TRN2", target_bir_lowering=False, debug=False,
                   enable_asserts=False, num_devices=ncores)

    # ---- DRAM I/O ----
    xbt_d = nc.dram_tensor("xbt", [B * CH, 128, DC * 512], BF16,
                           kind="ExternalInput").ap()
    x8t_d = nc.dram_tensor("x8t", [B * CH, 128, DC * 512], F8,
                           kind="ExternalInput").ap()
    wqk_d = nc.dram_tensor("wqk", [128, DC * 2 * HPC * KD], BF16,
                           kind="ExternalInput").ap()
    bqk_d = nc.dram_tensor("bqk", [1, 2 * HPC * KD], BF16,
                           kind="ExternalInput").ap()
    wv8_d = nc.dram_tensor("wv8", [HPC, 128, DC * D], F8,
                           kind="ExternalInput").ap()
    padsh_d = nc.dram_tensor("padsh", [B, S], BF16, kind="ExternalInput").ap()
    dmask_d = nc.dram_tensor("dmask", [128, 512], F32,
                             kind="ExternalInput").ap()
    bvsb_d = nc.dram_tensor("bvsb", [128, D], F32, kind="ExternalInput").ap()
    q1m_d = nc.dram_tensor("q1m", [128, 2 * PH], F32,
                           kind="ExternalInput").ap()
    qp_d = nc.dram_tensor("qp", [128, 2 * PH], F32, kind="ExternalInput").ap()
    xs_d = nc.dram_tensor("xs", [PH * QT, D], F32, kind="ExternalInput").ap()
    id_d = nc.dram_tensor("ident", [128, 128], F32, kind="ExternalInput").ap()
    w1_d = nc.dram_tensor("w1s", [FB, 128, D], BF16, kind="ExternalInput").ap()
    b1t_d = nc.dram_tensor("b1t", [128, FB], F32, kind="ExternalInput").ap()
    w2r_d = nc.dram_tensor("w2r", [128, FB * D], BF16,
                           kind="ExternalInput").ap()
    b2b_d = nc.dram_tensor("b2b", [128, D], F32, kind="ExternalInput").ap()
    ln1w_d = nc.dram_tensor("ln1w", [128, D], F32, kind="ExternalInput").ap()
    ln1b_d = nc.dram_tensor("ln1b", [128, D], F32, kind="ExternalInput").ap()
    ln2w_d = nc.dram_tensor("ln2w", [128, D], F32, kind="ExternalInput").ap()
    ln2b_d = nc.dram_tensor("ln2b", [128, D], F32, kind="ExternalInput").ap()
    out_d = nc.dram_tensor("out", [PH * QT, D], F32, kind="ExternalOutput").ap()

    with tile.TileContext(nc) as tc, ExitStack() as ctx0:
        dramp = ctx0.enter_context(tc.tile_pool(name="dram", bufs=1,
                                                space="DRAM"))
        consts = ctx0.enter_context(tc.tile_pool(name="const", bufs=1))
        smalls = ctx0.enter_context(tc.tile_pool(name="smalls", bufs=6))
        rowp = ctx0.enter_context(tc.tile_pool(name="rowp", bufs=1))
        w2pool = ctx0.enter_context(tc.tile_pool(name="w2p", bufs=1))
        attn_ctx = ExitStack()
        wvp = attn_ctx.enter_context(tc.tile_pool(name="wv", bufs=1))
        qkp = attn_ctx.enter_context(tc.tile_pool(name="qkt", bufs=2))
        v8p = attn_ctx.enter_context(tc.tile_pool(name="v8", bufs=2))
        e8p = attn_ctx.enter_context(tc.tile_pool(name="e8", bufs=10))
        osbp = attn_ctx.enter_context(tc.tile_pool(name="osb", bufs=3))
        xbcp = attn_ctx.enter_context(tc.tile_pool(name="xbc", bufs=2))
        x8cp = attn_ctx.enter_context(tc.tile_pool(name="x8c", bufs=2))

        o_exts = [dramp.tile([8 * SLICE, D], F32, name=f"oext{p}")
                  for p in range(PH)]
        rs_outs = [dramp.tile([SLICE, D], F32, name=f"rsout{p}")
                   for p in range(PH)]

        # ---- constants ----
        ident = consts.tile([128, 128], F32, tag="ident")
        nc.sync.dma_start(ident[:], id_d[:])
        dmask = consts.tile([128, 512], F32, tag="dmask")
        nc.sync.dma_start(dmask[:], dmask_d[:])
        bvsb = consts.tile([128, D], F32, tag="bvsb")
        nc.sync.dma_start(bvsb[:], bvsb_d[:])
        q1m = consts.tile([128, 2 * PH], F32, tag="q1m")
        nc.sync.dma_start(q1m[:], q1m_d[:])
        qp = consts.tile([128, 2 * PH], F32, tag="qp")
        nc.sync.dma_start(qp[:], qp_d[:])
        b1t = consts.tile([128, FB], F32, tag="b1t")
        nc.sync.dma_start(b1t[:], b1t_d[:])
        onesb = consts.tile([1, 512], BF16, tag="onesb")
        nc.vector.memset(onesb[:], 1.0)
        onesbc = consts.tile([1, 128], BF16, tag="onesbc")
        nc.vector.memset(onesbc[:], 1.0)

        # resident weights
        wqk = consts.tile([128, DC * 2 * HPC * KD], BF16, tag="wqk")
        nc.sync.dma_start(wqk[:], wqk_d[:])
        wqk3 = wqk[:].rearrange("p (c r) -> p c r", r=2 * HPC * KD)
        bqk = consts.tile([1, 2 * HPC * KD], BF16, tag="bqk")
        nc.sync.dma_start(bqk[:], bqk_d[:])
        wv8 = wvp.tile([128, HPC * DC * D], F8, tag="wv8")
        nc.sync.dma_start(
            wv8[:].rearrange("p (h r) -> p h r", h=HPC),
            wv8_d[:].rearrange("h p r -> p h r"))
        w2sb = w2pool.tile([128, FB * D], BF16, tag="w2sb")
        nc.sync.dma_start(w2sb[:], w2r_d[:])

        def layer_norm(x_ap, w_ap, b_ap, out_ap, stg_pool):
            G = D // 512
            st6 = smalls.tile([128, 6 * G], F32, tag="st6")
            for gg in range(G):
                nc.vector.bn_stats(st6[:, 6 * gg:6 * gg + 6],
                                   x_ap[:, 512 * gg:512 * gg + 512])
            mv = smalls.tile([128, 2], F32, tag="mv")
            nc.vector.bn_aggr(mv[:], st6[:])
            ve = smalls.tile([128, 1], F32, tag="ve")
            nc.vector.tensor_scalar_add(ve[:], mv[:, 1:2], EPS)
            sd = smalls.tile([128, 1], F32, tag="sd")
            nc.scalar.sqrt(sd[:], ve[:])
            rs_ = smalls.tile([128, 1], F32, tag="rs")
            nc.vector.reciprocal(rs_[:], sd[:])
            xc = stg_pool.tile([128, D], F32, tag="stg")
            nc.vector.tensor_scalar_sub(xc[:], x_ap, mv[:, 0:1])
            nc.vector.scalar_tensor_tensor(out_ap, xc[:], rs_[:], w_ap,
                                           ALU.mult, ALU.mult)
            nc.vector.tensor_add(out_ap, out_ap, b_ap)

        # tensor-engine backlog of deferred PV passes (emitted interleaved
        # with the next chunk of scores/exp so the in-order tensor stream
        # never stalls on the scalar engine)
        backlog = deque()

        def drain(n):
            for _ in range(n):
                if not backlog:
                    return
                backlog.popleft()()

        def attention_batch(b):
            # ---- SBUF tiles for this batch ----
            qkts = [qkp.tile([65, 2 * S], BF16, tag="qkt", name=f"qkt{h}")
                    for h in range(HPC)]
            v8s = [v8p.tile([128, VW], F8, tag="v8", name=f"v8_{h}")
                   for h in range(HPC)]

            def v8pair(h, pr):
                """AP [128, 2, 1025] for tb-pair pr of head h."""
                return v8s[h][:, pr * 2050:pr * 2050 + 2050].rearrange(
                    "p (j e) -> p j e", j=2)

            def wv8ap(h, c2):
                base = (h * DC + c2 * 2) * D
                return wv8[:, base:base + 2 * D].rearrange(
                    "p (j e) -> p j e", j=2)

            for h in range(HPC):
                nc.vector.memset(qkts[h][64:65, 0:S], 1.0)
                nc.sync.dma_start(qkts[h][64:65, S:2 * S], padsh_d[b:b + 1, :])
                nc.vector.memset(
                    v8s[h][:].rearrange("p (r e) -> p r e", e=1025)
                    [:, :, 1024:1025], 1.0)

            ps1 = ExitStack()
            pqk = ps1.enter_context(
                tc.tile_pool(name="pqk", bufs=2, space="PSUM"))
            pvv = ps1.enter_context(
                tc.tile_pool(name="pvv", bufs=2, space="PSUM"))
            pvr = ps1.enter_context(
                tc.tile_pool(name="pvr", bufs=1, space="PSUM"))

            # ---- QK projection for both heads (bf16) ----
            for c4 in range(CH):
                xbc = xbcp.tile([128, DC * 512], BF16, tag="xbc")
                nc.sync.dma_start(xbc[:], xbt_d[b * CH + c4])
                for h in range(HPC):
                    for w in range(2):
                        g = (h * 2 + w) * KD
                        ps = pqk.tile([64, 512], F32, tag="qk")
                        for c in range(DC):
                            nc.tensor.matmul(
                                ps[:], wqk3[:, c, g:g + KD],
                                xbc[:, c * 512:c * 512 + 512],
                                start=(c == 0), stop=False)
                        nc.tensor.matmul(
                            ps[:], bqk[0:1, g:g + KD], onesb[0:1, 0:512],
                            start=False, stop=True)
                        off = w * S + c4 * 512
                        nc.scalar.activation(
                            qkts[h][0:64, off:off + 512], ps[:], AF.Copy,
                            scale=(0.125 if w == 0 else 1.0))

            # ---- V projection fp8 DoubleRow; v includes a ones column ----
            vrps = pvr.tile([1, D], F32, tag="vr")
            xsc = smalls.tile([128, CH * DC], F32, tag="xsc")
            for ct in range(CH):
                x8c = x8cp.tile([128, DC * 512], F8, tag="x8c")
                nc.sync.dma_start(x8c[:], x8t_d[b * CH + ct])
                for c in range(DC):
                    with nc.allow_low_precision(reason="xsum stats"):
                        nc.vector.tensor_reduce(
                            xsc[:, ct * DC + c:ct * DC + c + 1],
                            x8c[:, c * 512:c * 512 + 512], AX.X, ALU.add)
                for h in range(HPC):
                    for ti in range(4):
                        tb = ct * 4 + ti
                        vps = pvv.tile([128, D], F32, tag="vv")
                        for c2 in range(DC // 2):
                            xap = x8c[:, c2 * 1024:c2 * 1024 + 1024].rearrange(
                                "p (j t) -> p j t", j=2)
                            for ec in range(EC):
                                nc.tensor.matmul(
                                    vps[:, ec * 512:ec * 512 + 512],
                                    xap[:, :, ti * 128:ti * 128 + 128],
                                    wv8ap(h, c2)[:, :, ec * 512:ec * 512 + 512],
                                    start=(c2 == 0), stop=(c2 == DC // 2 - 1),
                                    perf_mode=DR)
                        nc.scalar.activation(
                            v8pair(h, tb // 2)[:, tb % 2, 0:1024], vps[:],
                            AF.Copy, scale=1.0 / SV)
            # vmean row: (sum_t x) @ Wv via xsum in fp8, both heads summed
            xsum = smalls.tile([128, DC], F32, tag="xsum")
            nc.vector.tensor_add(xsum[:], xsc[:, 0:DC], xsc[:, DC:2 * DC])
            nc.vector.tensor_add(xsum[:], xsum[:], xsc[:, 2 * DC:3 * DC])
            nc.vector.tensor_add(xsum[:], xsum[:], xsc[:, 3 * DC:4 * DC])
            xsum8 = smalls.tile([128, DC], F8, tag="xsum8")
            nc.scalar.activation(xsum8[:], xsum[:], AF.Copy, scale=1.0 / SX)
            for h in range(HPC):
                for c in range(DC):
                    base = (h * DC + c) * D
                    for ec in range(EC):
                        nc.tensor.matmul(
                            vrps[:, ec * 512:ec * 512 + 512],
                            xsum8[:, c:c + 1],
                            wv8[:, base + ec * 512:base + ec * 512 + 512],
                            start=(h == 0 and c == 0),
                            stop=(h == HPC - 1 and c == DC - 1))
            vrow = rowp.tile([1, D], F32, tag="vrow")
            nc.vector.scalar_tensor_tensor(vrow[:], vrps[:], SX / (SV * S),
                                           bvsb[0:1, :], ALU.mult, ALU.add)
            for sl in range(8):
                nc.gpsimd.dma_start(
                    o_exts[b][sl * SLICE + 256:sl * SLICE + 257, :], vrow[:])
            ps1.close()

            ps2 = ExitStack()
            pst = ps2.enter_context(
                tc.tile_pool(name="pst", bufs=1, space="PSUM"))
            pops = ps2.enter_context(
                tc.tile_pool(name="pops", bufs=2, space="PSUM"))

            # ---- attention over query chunks ----
            def pv_unit(h, s, ic, e8list):
                """Deferred PV accumulation pass for (h, s) over e8 pairs."""
                ops = [None]

                def mk(p):
                    def f():
                        if p == 0:
                            ops[0] = pops.tile([128, 1025], F32, tag="ops",
                                               name=f"ops{ic}_{h}{s}")
                        e8ap = e8list[p][:, h * 512:h * 512 + 512].rearrange(
                            "p (j q) -> p j q", j=2)
                        est = e8ap[:, :, s * 128:s * 128 + 128]
                        vp = v8pair(h, p)
                        for ec in range(EC):
                            nc.tensor.matmul(
                                ops[0][:, ec * 512:ec * 512 + 512], est,
                                vp[:, :, ec * 512:ec * 512 + 512],
                                start=(p == 0), stop=(p == ic), perf_mode=DR)
                        nc.tensor.matmul(
                            ops[0][:, 1024:1025], est, vp[:, :, 1024:1025],
                            start=(p == 0), stop=(p == ic), perf_mode=DR)
                    return f

                def fin():
                    dsb = smalls.tile([128, 1], F32, tag="dsb")
                    nc.vector.tensor_scalar_add(dsb[:], ops[0][:, 1024:1025],
                                                1e-30)
                    rd = smalls.tile([128, 1], F32, tag="rd")
                    nc.vector.reciprocal(rd[:], dsb[:])
                    if h == 0:
                        t = osbp.tile([128, D], F32, tag="osb")
                        nc.vector.scalar_tensor_tensor(
                            t[:], ops[0][:, 0:1024], rd[:], bvsb[:],
                            ALU.mult, ALU.add)
                        tmp_s[(ic, s)] = t
                    else:
                        osb = osbp.tile([128, D], F32, tag="osb")
                        nc.vector.scalar_tensor_tensor(
                            osb[:], ops[0][:, 0:1024], rd[:],
                            tmp_s.pop((ic, s))[:], ALU.mult, ALU.add)
                        row = ic * SLICE + s * 128
                        nc.gpsimd.dma_start(
                            o_exts[b][row:row + 128, :], osb[:])
                return [mk(p) for p in range(ic + 1)] + [fin]

            tmp_s = {}
            for ic in range(IC):
                e8list = []
                for p in range(ic + 1):
                    st = pst.tile([128, 1024], F32, tag="st")
                    for h in range(HPC):
                        for j in range(2):
                            tb = 2 * p + j
                            nc.tensor.matmul(
                                st[:, h * 512 + j * 256:h * 512 + j * 256 + 256],
                                qkts[h][:, S + tb * 128:S + tb * 128 + 128],
                                qkts[h][:, ic * 256:ic * 256 + 256],
                                start=True, stop=True)
                    if p == ic:
                        for h in range(HPC):
                            nc.vector.tensor_add(
                                st[:, h * 512:h * 512 + 512],
                                st[:, h * 512:h * 512 + 512], dmask[:])
                    e8 = e8p.tile([128, 1024], F8, tag="e8")
                    nc.scalar.activation(e8[:], st[:], AF.Exp)
                    e8list.append(e8)
                    # PV pass A (h0, s0) rides along; drain some backlog
                    drain(3)
                # h0 s0 unit runs right away (interleaved above via drain);
                # actually emit it now, then queue the other three.
                for f in pv_unit(0, 0, ic, e8list):
                    f()
                for (h, s) in ((0, 1), (1, 0), (1, 1)):
                    backlog.extend(pv_unit(h, s, ic, e8list))
                drain(4)
            drain(len(backlog))
            ps2.close()

        # ======== emission: b0 RS0 b1 RS1 b2 RS2 b3 q0 q1 q2 RS3 q3 ========
        def rs_phase(p):
            nc.gpsimd.collective_compute(
                "ReduceScatter", ALU.add,
                replica_groups=[list(range(ncores))],
                ins=[o_exts[p].opt()], outs=[rs_outs[p].opt()])

        def ffn_quarter(q):
            with ExitStack() as fs:
                stg = fs.enter_context(tc.tile_pool(name="stg", bufs=4))
                vtp = fs.enter_context(tc.tile_pool(name="vtp", bufs=1))
                x1p = fs.enter_context(tc.tile_pool(name="x1", bufs=2))
                x1tp = fs.enter_context(tc.tile_pool(name="x1t", bufs=1))
                htp = fs.enter_context(tc.tile_pool(name="hts", bufs=1))
                w1p = fs.enter_context(tc.tile_pool(name="w1s", bufs=4))
                ptp = fs.enter_context(tc.tile_pool(name="ptp", bufs=1,
                                                    space="PSUM"))
                php = fs.enter_context(tc.tile_pool(name="php", bufs=2,
                                                    space="PSUM"))
                pyp = fs.enter_context(tc.tile_pool(name="pyp", bufs=2,
                                                    space="PSUM"))
                rsrc = rs_outs[q]
                # vtb: broadcast vmean row
                vtr = rowp.tile([1, D], F32, tag="vtr")
                nc.sync.dma_start(vtr[:], rsrc[256:257, :])
                vtrb = rowp.tile([1, D], BF16, tag="vtrb")
                nc.scalar.copy(vtrb[:], vtr[:])
                bps = ptp.tile([128, D], F32, tag="ptp")
                for ec in range(EC):
                    nc.tensor.matmul(bps[:, ec * 512:ec * 512 + 512],
                                     onesbc[0:1, :],
                                     vtrb[0:1, ec * 512:ec * 512 + 512],
                                     start=True, stop=True)
                vtb = vtp.tile([128, D], F32, tag="vtb")
                nc.vector.tensor_copy(out=vtb[:], in_=bps[:])

                x1s = []
                for tl in range(TBH):
                    sb = q * TBH + tl
                    rsb = stg.tile([128, D], F32, tag="stg")
                    nc.sync.dma_start(rsb[:], rsrc[tl * 128:tl * 128 + 128, :])
                    xsb = stg.tile([128, D], F32, tag="stg")
                    nc.sync.dma_start(
                        xsb[:], xs_d[sb * 128:sb * 128 + 128, :])
                    t0 = stg.tile([128, D], F32, tag="stg")
                    nc.vector.scalar_tensor_tensor(
                        t0[:], rsb[:], q1m[:, sb:sb + 1], xsb[:],
                        ALU.mult, ALU.add)
                    x0 = stg.tile([128, D], F32, tag="stg")
                    nc.vector.scalar_tensor_tensor(
                        x0[:], vtb[:], qp[:, sb:sb + 1], t0[:],
                        ALU.mult, ALU.add)
                    x1 = x1p.tile([128, D], F32, tag="x1")
                    layer_norm(x0[:], fc["ln1w"][:], fc["ln1b"][:],
                               x1[:], stg)
                    x1s.append(x1)

                # x1^T in bf16
                x1t = x1tp.tile([128, DC * QT], BF16, tag="x1t")
                x1tr = x1t[:].rearrange("p (c t) -> p c t", c=DC)
                for tl in range(TBH):
                    tp = ptp.tile([128, D], F32, tag="ptp")
                    for c in range(DC):
                        nc.tensor.transpose(
                            tp[:, c * 128:c * 128 + 128],
                            x1s[tl][:, c * 128:c * 128 + 128], ident[:])
                    nc.vector.tensor_copy(
                        out=x1tr[:, :, tl * 128:tl * 128 + 128],
                        in_=tp[:].rearrange("p (c t) -> p c t", c=DC))

                # hT = relu(W1^T x1^T + b1)
                hts = htp.tile([128, FB * QT], BF16, tag="hts")
                for fb in range(FB):
                    w1s = w1p.tile([128, D], BF16, tag="w1s")
                    nc.sync.dma_start(w1s[:], w1_d[fb])
                    hps = php.tile([128, QT], F32, tag="php")
                    for c in range(DC):
                        nc.tensor.matmul(hps[:], w1s[:, c * 128:c * 128 + 128],
                                         x1t[:, c * QT:(c + 1) * QT],
                                         start=(c == 0), stop=(c == DC - 1))
                    nc.scalar.activation(hts[:, fb * QT:(fb + 1) * QT],
                                         hps[:], AF.Relu,
                                         bias=b1t[:, fb:fb + 1], scale=1.0)

                # y = hT.T @ W2 (full accumulation in psum)
                for tl in range(TBH):
                    yps = pyp.tile([128, D], F32, tag="pyp")
                    for fb in range(FB):
                        for ec in range(EC):
                            nc.tensor.matmul(
                                yps[:, ec * 512:ec * 512 + 512],
                                hts[:, fb * QT + tl * 128:
                                    fb * QT + tl * 128 + 128],
                                w2sb[:, fb * D + ec * 512:fb * D + ec * 512 + 512],
                                start=(fb == 0), stop=(fb == FB - 1))
                    x2 = stg.tile([128, D], F32, tag="stg")
                    nc.vector.scalar_tensor_tensor(
                        x2[:], yps[:], 1.0, x1s[tl][:], ALU.mult, ALU.add)
                    nc.vector.tensor_add(x2[:], x2[:], fc["b2b"][:])
                    ot = stg.tile([128, D], F32, tag="stg")
                    layer_norm(x2[:], fc["ln2w"][:], fc["ln2b"][:],
                               ot[:], stg)
                    row = (q * TBH + tl) * 128
                    nc.sync.dma_start(out_d[row:row + 128, :], ot[:])

        fc = {}
        for b in range(B):
            attention_batch(b)
            if b < B - 1:
                rs_phase(b)
        attn_ctx.close()
        fcon = ctx0.enter_context(tc.tile_pool(name="fcon", bufs=1))
        for nm, dd in [("ln1w", ln1w_d), ("ln1b", ln1b_d),
                       ("ln2w", ln2w_d), ("ln2b", ln2b_d), ("b2b", b2b_d)]:
            t = fcon.tile([128, D], F32, tag=nm, name=nm)
            nc.sync.dma_start(t[:], dd[:])
            fc[nm] = t
        for q in range(3):
            ffn_quarter(q)
        rs_phase(3)
        ffn_quarter(3)

    nc.compile()
    return nc


# ------------------------- host side -------------------------

_NC_CACHE = {}


def _get_nc(cfg_key):
    if cfg_key not in _NC_CACHE:
        _NC_CACHE[cfg_key] = build_nc(**CFG_MAIN)
    return _NC_CACHE[cfg_key]


def make_in_maps(inputs, B, S, D, FF, ncores, HPC):
    DC = D // 128
    FB = FF // 128
    H = ncores * HPC
    PH = B
    QT = 256
    CH = S // 512
    bf = ml_dtypes.bfloat16
    f8 = ml_dtypes.float8_e4m3

    x = np.ascontiguousarray(
        np.asarray(inputs["input"], dtype=np.float32).reshape(B, S, D))
    pad = np.asarray(inputs["padding_mask"], dtype=bool)
    Wq = np.asarray(inputs["Wq"], dtype=np.float32)
    Wk = np.asarray(inputs["Wk"], dtype=np.float32)
    Wv = np.asarray(inputs["Wv"], dtype=np.float32)
    bq = np.asarray(inputs["bq"], dtype=np.float32)
    bk = np.asarray(inputs["bk"], dtype=np.float32)
    bvv = np.asarray(inputs["bv"], dtype=np.float32)

    # transposed x: [B*CH, 128, DC*512]; row p col (c*512+ti) of chunk
    # (b,c4) = x[b, c4*512+ti, c*128+p]
    xt = x.reshape(B, CH, 512, DC, 128).transpose(0, 1, 4, 3, 2)
    xbt = np.ascontiguousarray(xt).astype(bf).reshape(B * CH, 128, DC * 512)
    x8t = np.asarray(xbt, np.float32).astype(f8)

    padsh = (np.where(pad, np.float32(NEG), np.float32(0.0))
             - np.float32(SHIFT)).astype(bf)

    cmask = np.zeros((128, 128), dtype=np.float32)
    cmask[np.tril_indices(128, -1)] = NEG
    dmask = np.zeros((128, 512), dtype=np.float32)
    dmask[:, 0:128] = cmask            # j0, s0 (diag)
    dmask[:, 256:384] = NEG            # j1, s0 (fully masked)
    dmask[:, 384:512] = cmask          # j1, s1 (diag)

    w1 = np.asarray(inputs["ff1_w"], dtype=np.float32)
    w1s = np.ascontiguousarray(
        w1.reshape(DC, 128, FB, 128).transpose(2, 1, 0, 3)
        .reshape(FB, 128, D)).astype(bf)
    w2 = np.asarray(inputs["ff2_w"], dtype=np.float32)
    w2r = np.ascontiguousarray(
        w2.reshape(FB, 128, D).transpose(1, 0, 2).reshape(128, FB * D)
    ).astype(bf)
    b1 = np.asarray(inputs["ff1_b"], dtype=np.float32)
    b1t = np.ascontiguousarray(b1.reshape(FB, 128).T)
    b2b = np.ascontiguousarray(
        np.broadcast_to(np.asarray(inputs["ff2_b"], np.float32), (128, D)))

    def bc(name):
        return np.ascontiguousarray(np.broadcast_to(
            np.asarray(inputs[name], np.float32), (128, D)))

    ident = np.eye(128, dtype=np.float32)
    xflat = x.reshape(B * S, D)
    padflat = pad.reshape(B * S)

    in_maps = []
    for c in range(ncores):
        h0 = c * HPC
        # wqk packing: [128, c, (h,w,KD)]
        wqk = np.empty((128, DC, 2 * HPC * KD), np.float32)
        for h in range(HPC):
            for w, W in ((0, Wq), (1, Wk)):
                g = (h * 2 + w) * KD
                wqk[:, :, g:g + KD] = (
                    W[h0 + h].reshape(DC, 128, KD).transpose(1, 0, 2))
        bqk = np.concatenate(
            [np.stack([bq[h0 + h], bk[h0 + h]]).reshape(-1)
             for h in range(HPC)]).reshape(1, -1)
        # wv8: [h, p, (c2, j, e)] = 32*Wv[h0+h, (2*c2+j)*128+p, e]
        wv8 = (Wv[h0:h0 + HPC] * SV).reshape(HPC, DC, 128, D) \
            .transpose(0, 2, 1, 3).reshape(HPC, 128, DC * D)
        bvs = np.broadcast_to(
            (bvv[h0] + bvv[h0 + 1]).astype(np.float32), (128, D))

        # this core's FFN tokens: phase p -> batch p tokens [256c, 256c+256)
        tok_idx = np.concatenate([
            np.arange(p * S + QT * c, p * S + QT * c + QT) for p in range(PH)])
        prow = padflat[tok_idx].reshape(2 * PH, 128).T.astype(np.float32)

        m = {
            "xbt": xbt,
            "x8t": x8t,
            "wqk": np.ascontiguousarray(
                wqk.reshape(128, DC * 2 * HPC * KD)).astype(bf),
            "bqk": bqk.astype(bf),
            "wv8": np.ascontiguousarray(wv8).astype(f8),
            "padsh": padsh,
            "dmask": dmask,
            "bvsb": np.ascontiguousarray(bvs),
            "q1m": np.ascontiguousarray((1.0 - prow) / H),
            "qp": np.ascontiguousarray(prow / H),
            "xs": np.ascontiguousarray(xflat[tok_idx]),
            "ident": ident,
            "w1s": w1s,
            "b1t": b1t,
            "w2r": w2r,
            "b2b": b2b,
            "ln1w": bc("ln1_w"),
            "ln1b": bc("ln1_b"),
            "ln2w": bc("ln2_w"),
            "ln2b": bc("ln2_b"),
        }
        in_maps.append(m)
    return in_maps


def gather_out(results, B, S, D, ncores):
    QT = 256
    PH = B
    out = np.empty((B * S, D), dtype=np.float32)
    for c in range(ncores):
        r_ = np.asarray(results[c]["out"])
        for p in range(PH):
            g0 = p * S + QT * c
            out[g0:g0 + QT] = r_[p * QT:(p + 1) * QT]
    return out.reshape(B, S, D)


def kernel(**inputs):
    from concourse.bass_utils import run_bass_kernel_spmd
    cfg = CFG_MAIN
    B, S, D = cfg["B"], cfg["S"], cfg["D"]
    ncores = cfg["ncores"]
    nc = _get_nc("main")
    in_maps = make_in_maps(inputs, **cfg)
    res = run_bass_kernel_spmd(nc, in_maps, list(range(ncores)))
    return gather_out(res.results, B, S, D, ncores).astype(np.float32)
